# revision 35
# baseline (speedup 1.0000x reference)
"""Causal multi-head attention block on 8 trn2 NeuronCores.

Tensor-parallel over heads: core c handles heads 2c, 2c+1 (for both batch
rows), computing q/k/v projections for its 128 channels, causal softmax
attention, and a partial output projection against its 128 rows of W_proj.
The host sums the 8 partial outputs and adds b_proj.

Layout strategy: everything on-chip lives in "transposed" space. The host
supplies x^T [B, C, T]; Q^T/K^T/V^T [128, T] come from matmuls with the
weight chunk as the stationary operand. V is PE-transposed back to [T, 128]
and augmented with a ones column so the P@V matmul also yields the softmax
denominator. Scores are computed as S^T = (K^T)^T @ Q^T with the two heads
row-packed on the PE array (each K=64). Causal masking adds a -60 constant
via an extra accumulating matmul (identity @ mask). All matmuls are fp32r
with free dim 512 (full-rate). Normalization is a broadcast-DMA of the
denominator row + reciprocal + multiply fused into the y^T copy.
"""

import math

import numpy as np

B, T, C, H = 2, 2048, 1024, 16
HS = C // H  # 64 head size
NCORES = 8
HPC = H // NCORES  # 2 heads per core
CPC = HPC * HS  # 128 channels per core
QB = 512  # q block width
NQB = T // QB  # 4
NKC = T // 128  # 16 k-chunks
CCH = C // 128  # 8 contraction chunks
NTB = T // 512  # 4 t-blocks for QKV
MASK_NEG = -60.0

_PROGRAM = None


def _build_program():
    import concourse.bass as bass
    import concourse.tile as tile
    from concourse import bacc, mybir

    f32 = mybir.dt.float32
    f32r = mybir.dt.float32r
    bf16 = mybir.dt.bfloat16
    EXP = mybir.ActivationFunctionType.Exp
    LN = mybir.ActivationFunctionType.Ln

    nc = bacc.Bacc("TRN2", target_bir_lowering=False, debug=False,
                   num_devices=NCORES)

    xT_d = nc.dram_tensor("xT", [B, C, T], bf16, kind="ExternalInput")
    wq_d = nc.dram_tensor("wq", [C, CPC], bf16, kind="ExternalInput")
    wk_d = nc.dram_tensor("wk", [C, CPC], bf16, kind="ExternalInput")
    wv_d = nc.dram_tensor("wv", [C, CPC], bf16, kind="ExternalInput")
    wp_d = nc.dram_tensor("wp", [CPC, C], bf16, kind="ExternalInput")
    ident_d = nc.dram_tensor("ident", [128, 128], bf16, kind="ExternalInput")
    tril_d = nc.dram_tensor("tril", [128, 128], bf16, kind="ExternalInput")
    ones_d = nc.dram_tensor("ones", [128, 128], f32r, kind="ExternalInput")
    bsel_d = nc.dram_tensor("bsel", [65, 128], f32r, kind="ExternalInput")
    out_d = nc.dram_tensor("out", [B, T, C], f32, kind="ExternalOutput")

    with tile.TileContext(nc) as tc:
        with (
            tc.tile_pool(name="static", bufs=1) as static,
            tc.tile_pool(name="xt", bufs=20) as xtp,
            tc.tile_pool(name="qt", bufs=10) as qtp,
            tc.tile_pool(name="kt", bufs=10) as ktp,
            tc.tile_pool(name="vt", bufs=5) as vtp,
            tc.tile_pool(name="vh0", bufs=34) as vh0p,
            tc.tile_pool(name="vh1", bufs=34) as vh1p,
            tc.tile_pool(name="et", bufs=4) as etp,
            tc.tile_pool(name="yt", bufs=4) as ytp,
            tc.tile_pool(name="dst", bufs=3) as dstp,
            tc.tile_pool(name="rf", bufs=6) as rfp,
            tc.tile_pool(name="osb", bufs=8) as osbp,
            tc.tile_pool(name="mm", bufs=2, space="PSUM") as mmps,
            tc.tile_pool(name="sp", bufs=4, space="PSUM") as sps,
            tc.tile_pool(name="av", bufs=2, space="PSUM") as avps,
        ):
            # ---- static loads ----
            wq_sb = static.tile([128, CCH, CPC], bf16, tag="wq")
            wk_sb = static.tile([128, CCH, CPC], bf16, tag="wk")
            wv_sb = static.tile([128, CCH, CPC], bf16, tag="wv")
            for w_sb, w_d in ((wq_sb, wq_d), (wk_sb, wk_d), (wv_sb, wv_d)):
                nc.sync.dma_start(
                    out=w_sb[:],
                    in_=w_d.ap().rearrange("(cc p) d -> p cc d", p=128),
                )
            wp_sb = static.tile([128, C], bf16, tag="wp")
            nc.sync.dma_start(out=wp_sb[:], in_=wp_d[:])
            tril_sb = static.tile([128, 128], bf16, tag="tril")
            nc.sync.dma_start(out=tril_sb[:], in_=tril_d[:])
            ident_sb = static.tile([128, 128], bf16, tag="ident")
            nc.sync.dma_start(out=ident_sb[:], in_=ident_d[:])
            ones_sb = static.tile([128, 128], f32r, tag="ones")
            nc.sync.dma_start(out=ones_sb[:], in_=ones_d[:])
            bsel_sb = static.tile([65, 128], f32r, tag="bsel")
            nc.sync.dma_start(out=bsel_sb[:], in_=bsel_d[:])

            def phase_a_alloc():
                st = {}
                st["qts"], st["kts"] = [], []
                for _tb in range(NTB):
                    qt_t = qtp.tile([128, 512], bf16, tag="qt", name="qt_t")
                    kt_t = ktp.tile([128, 512], bf16, tag="kt", name="kt_t")
                    st["qts"].append(qt_t)
                    st["kts"].append(kt_t)
                st["vh0s"], st["vh1s"] = [], []
                for _kc in range(NKC):
                    vh0_t = vh0p.tile([128, 65], bf16, tag="vh0",
                                      name="vh0_t")
                    vh1_t = vh1p.tile([128, 128], bf16, tag="vh1",
                                      name="vh1_t")
                    # ones columns for the denominator trick
                    nc.vector.tensor_copy(vh0_t[:, 64:65], ones_sb[:, 0:1])
                    nc.vector.tensor_copy(vh1_t[:, 0:1], ones_sb[:, 0:1])
                    st["vh0s"].append(vh0_t)
                    st["vh1s"].append(vh1_t)
                return st

            def phase_a_tb(st, b, tb):
                qts, kts = st["qts"], st["kts"]
                vh0s, vh1s = st["vh0s"], st["vh1s"]
                xts = []
                for cc in range(CCH):
                    xt_t = xtp.tile([128, 512], bf16, tag="xt", name="xt_t")
                    nc.sync.dma_start(
                        out=xt_t[:],
                        in_=xT_d[b, cc * 128:(cc + 1) * 128,
                                 tb * 512:(tb + 1) * 512],
                    )
                    xts.append(xt_t)
                for which, w_sb in (("q", wq_sb), ("k", wk_sb),
                                    ("v", wv_sb)):
                    ps = mmps.tile([128, 512], f32, tag="mm", name="ps")
                    for cc in range(CCH):
                        nc.tensor.matmul(
                            ps[:], w_sb[:, cc, :], xts[cc][:],
                            start=(cc == 0), stop=(cc == CCH - 1),
                        )
                    if which == "q":
                        nc.vector.tensor_copy(qts[tb][:], ps[:])
                    elif which == "k":
                        nc.vector.tensor_copy(kts[tb][:], ps[:])
                    else:
                        vt_sb = vtp.tile([128, 512], bf16, tag="vt",
                                         name="vt_sb")
                        nc.vector.tensor_copy(vt_sb[:], ps[:])
                        for j in range(4):
                            kc = tb * 4 + j
                            tps = mmps.tile([128, 128], bf16, tag="mm",
                                            name="tps")
                            nc.tensor.transpose(
                                tps[:], vt_sb[:, j * 128:(j + 1) * 128],
                                ident_sb[:])
                            nc.vector.tensor_copy(
                                vh0s[kc][:, 0:64], tps[:, 0:64])
                            nc.vector.tensor_copy(
                                vh1s[kc][:, 64:128], tps[:, 64:128])

            def phase_b_qb(st, b, qb):
                qts, kts = st["qts"], st["kts"]
                vh0s, vh1s = st["vh0s"], st["vh1s"]
                yt_sb = ytp.tile([128, 512], bf16, tag="yt", name="yt_sb")
                nkc = 4 * qb + 4
                av0 = avps.tile([128, QB], f32, tag="av", name="av0")
                av1 = avps.tile([128, QB], f32, tag="av", name="av1")
                av = [av0, av1]
                for kc in range(nkc):
                    m = kc - 4 * qb  # >= 0 on diagonal chunks
                    lo = 0 if m < 0 else 128 * m
                    for h in range(2):
                        st_ps = sps.tile([128, QB], f32, tag="s",
                                         name="st_ps")
                        nc.tensor.matmul(
                            st_ps[:, lo:QB],
                            kts[kc // 4][64 * h:64 * h + 64,
                                         (kc % 4) * 128:
                                         (kc % 4) * 128 + 128],
                            qts[qb][64 * h:64 * h + 64, lo:QB],
                            start=True, stop=True,
                        )
                        et = etp.tile([128, QB], bf16, tag="et", name="et")
                        nc.scalar.activation(et[:, lo:QB], st_ps[:, lo:QB],
                                             EXP)
                        if m >= 0:
                            nc.vector.tensor_mul(
                                et[:, lo:lo + 128], et[:, lo:lo + 128],
                                tril_sb[:])
                        avt = av[0][0:65, lo:QB] if h == 0 \
                            else av[1][:, lo:QB]
                        lhsT = vh0s[kc][:] if h == 0 else vh1s[kc][:]
                        nc.tensor.matmul(
                            avt, lhsT, et[:, lo:QB],
                            start=(kc == 0), stop=(kc == nkc - 1),
                        )
                # denominators -> reciprocal -> normalized y^T
                dst = dstp.tile([65, QB], f32r, tag="dst", name="dst")
                nc.vector.tensor_copy(dst[64:65, :], av[0][64:65, :])
                nc.vector.tensor_copy(dst[0:1, :], av[1][0:1, :])
                bps = sps.tile([128, QB], f32, tag="s", name="bps")
                nc.tensor.matmul(bps[:], bsel_sb[:], dst[:, :],
                                 start=True, stop=True)
                # copy unnormalized y^T out right away so the AV PSUM
                # banks release before the reciprocal chain completes
                nc.vector.tensor_copy(yt_sb[0:64, :], av[0][0:64, :])
                nc.vector.tensor_copy(yt_sb[64:128, :], av[1][64:128, :])
                rf = rfp.tile([128, QB], f32, tag="rf", name="rf")
                scr = rfp.tile([128, QB], f32, tag="scr", name="scr")
                nc.vector.reciprocal_approx_accurate(rf[:], bps[:], scr[:])
                for h in range(2):
                    sl = yt_sb[64 * h:64 * h + 64, :]
                    nc.vector.tensor_mul(sl, sl,
                                         rf[64 * h:64 * h + 64, :])
                # output projection for this q-block's four t-chunks
                for j in range(4):
                    tchunk = qb * 4 + j
                    for n in range(2):
                        pps = mmps.tile([128, 512], f32, tag="mm",
                                        name="pps")
                        nc.tensor.matmul(
                            pps[:],
                            yt_sb[:, j * 128:(j + 1) * 128],
                            wp_sb[:, n * 512:(n + 1) * 512],
                            start=True, stop=True,
                        )
                        osb = osbp.tile([128, 512], f32, tag="osb",
                                        name="osb")
                        nc.vector.tensor_copy(osb[:], pps[:])
                        nc.sync.dma_start(
                            out=out_d[b, tchunk * 128:(tchunk + 1) * 128,
                                      n * 512:(n + 1) * 512],
                            in_=osb[:],
                        )

            # batch 0 QKV, then batch 0 attention interleaved with
            # batch 1 QKV (one t-block per q-block) to keep PE dense
            st0 = phase_a_alloc()
            for tb in range(NTB):
                phase_a_tb(st0, 0, tb)
            st1 = phase_a_alloc()
            for qb in range(NQB):
                phase_b_qb(st0, 0, qb)
                phase_a_tb(st1, 1, qb)
            for qb in range(NQB):
                phase_b_qb(st1, 1, qb)

    nc.compile()
    return nc


def _get_program():
    global _PROGRAM
    if _PROGRAM is None:
        _PROGRAM = _build_program()
    return _PROGRAM


def _make_in_maps(x, W_kqv, W_proj):
    import ml_dtypes

    bf = ml_dtypes.bfloat16
    xT = np.ascontiguousarray(x.transpose(0, 2, 1)).astype(bf)
    scale = 1.0 / math.sqrt(HS)

    ident = np.eye(128, dtype=np.float32).astype(bf)
    bsel = np.zeros((65, 128), dtype=np.float32)
    bsel[64, 0:64] = 1.0
    bsel[0, 64:128] = 1.0

    in_maps = []
    for c in range(NCORES):
        lo, hi = CPC * c, CPC * (c + 1)
        in_maps.append({
            "xT": xT,
            "wq": (np.ascontiguousarray(W_kqv[:, lo:hi]) * scale).astype(bf),
            "wk": np.ascontiguousarray(W_kqv[:, C + lo:C + hi]).astype(bf),
            "wv": np.ascontiguousarray(W_kqv[:, 2 * C + lo:2 * C + hi]).astype(bf),
            "wp": np.ascontiguousarray(W_proj[lo:hi, :]).astype(bf),
            "ident": ident,
            "tril": np.triu(np.ones((128, 128), dtype=np.float32)).astype(bf),
            "ones": np.ones((128, 128), dtype=np.float32),
            "bsel": bsel,
        })
    return in_maps


def run(x, W_kqv, W_proj, b_proj, trace=False, trace_cores=None):
    from concourse.bass_utils import run_bass_kernel_spmd

    nc = _get_program()
    in_maps = _make_in_maps(np.asarray(x, dtype=np.float32),
                            np.asarray(W_kqv, dtype=np.float32),
                            np.asarray(W_proj, dtype=np.float32))
    res = run_bass_kernel_spmd(
        nc, in_maps, list(range(NCORES)),
        trace=trace, trace_cores=trace_cores,
    )
    out = np.zeros((B, T, C), dtype=np.float32)
    for c in range(NCORES):
        out += res.results[c]["out"]
    out += np.asarray(b_proj, dtype=np.float32)
    return out, res


def kernel(x, W_kqv, W_proj, b_proj):
    out, _ = run(x, W_kqv, W_proj, b_proj)
    return out


# revision 36
# speedup vs baseline: 1.1704x; 1.1704x over previous
"""Causal multi-head attention block on 8 trn2 NeuronCores.

Tensor-parallel over heads: core c handles heads 2c, 2c+1 (for both batch
rows), computing q/k/v projections for its 128 channels, causal softmax
attention, and a partial output projection against its 128 rows of W_proj.
The host sums the 8 partial outputs and adds b_proj.

Layout strategy: everything on-chip lives in "transposed" space. The host
supplies x^T [B, C, T]; Q^T/K^T/V^T [128, T] come from matmuls with the
weight chunk as the stationary operand. V is PE-transposed back to [T, 128]
and augmented with a ones column so the P@V matmul also yields the softmax
denominator. Scores are computed as S^T = (K^T)^T @ Q^T with the two heads
row-packed on the PE array (each K=64). Causal masking adds a -60 constant
via an extra accumulating matmul (identity @ mask). All matmuls are fp32r
with free dim 512 (full-rate). Normalization is a broadcast-DMA of the
denominator row + reciprocal + multiply fused into the y^T copy.
"""

import math

import numpy as np

B, T, C, H = 2, 2048, 1024, 16
HS = C // H  # 64 head size
NCORES = 8
HPC = H // NCORES  # 2 heads per core
CPC = HPC * HS  # 128 channels per core
QB = 512  # q block width
NQB = T // QB  # 4
NKC = T // 128  # 16 k-chunks
CCH = C // 128  # 8 contraction chunks
NTB = T // 512  # 4 t-blocks for QKV
MASK_NEG = -60.0

_PROGRAM = None


def _build_program():
    import concourse.bass as bass
    import concourse.tile as tile
    from concourse import bacc, mybir

    f32 = mybir.dt.float32
    f32r = mybir.dt.float32r
    bf16 = mybir.dt.bfloat16
    EXP = mybir.ActivationFunctionType.Exp
    LN = mybir.ActivationFunctionType.Ln

    nc = bacc.Bacc("TRN2", target_bir_lowering=False, debug=False,
                   num_devices=NCORES)

    xT_d = nc.dram_tensor("xT", [B, C, T], bf16, kind="ExternalInput")
    wq_d = nc.dram_tensor("wq", [C, CPC], bf16, kind="ExternalInput")
    wk_d = nc.dram_tensor("wk", [C, CPC], bf16, kind="ExternalInput")
    wv_d = nc.dram_tensor("wv", [C, CPC], bf16, kind="ExternalInput")
    wp_d = nc.dram_tensor("wp", [CPC, C], bf16, kind="ExternalInput")
    ident_d = nc.dram_tensor("ident", [128, 128], bf16, kind="ExternalInput")
    tril_d = nc.dram_tensor("tril", [128, 128], bf16, kind="ExternalInput")
    ones_d = nc.dram_tensor("ones", [128, 128], f32r, kind="ExternalInput")
    bsel_d = nc.dram_tensor("bsel", [65, 128], f32r, kind="ExternalInput")
    out_d = nc.dram_tensor("out", [B, T, C], f32, kind="ExternalOutput")

    with tile.TileContext(nc) as tc:
        with (
            tc.tile_pool(name="static", bufs=1) as static,
            tc.tile_pool(name="xt", bufs=16) as xtp,
            tc.tile_pool(name="qt", bufs=10) as qtp,
            tc.tile_pool(name="kt", bufs=10) as ktp,
            tc.tile_pool(name="vt", bufs=3) as vtp,
            tc.tile_pool(name="vh0", bufs=34) as vh0p,
            tc.tile_pool(name="vh1", bufs=34) as vh1p,
            tc.tile_pool(name="et", bufs=4) as etp,
            tc.tile_pool(name="yt", bufs=3) as ytp,
            tc.tile_pool(name="dst", bufs=3) as dstp,
            tc.tile_pool(name="rf", bufs=4) as rfp,
            tc.tile_pool(name="osb", bufs=6) as osbp,
            tc.tile_pool(name="mm", bufs=2, space="PSUM") as mmps,
            tc.tile_pool(name="sp", bufs=4, space="PSUM") as sps,
            tc.tile_pool(name="av", bufs=2, space="PSUM") as avps,
        ):
            # ---- static loads ----
            wq_sb = static.tile([128, CCH, CPC], bf16, tag="wq")
            wk_sb = static.tile([128, CCH, CPC], bf16, tag="wk")
            wv_sb = static.tile([128, CCH, CPC], bf16, tag="wv")
            for w_sb, w_d in ((wq_sb, wq_d), (wk_sb, wk_d), (wv_sb, wv_d)):
                nc.sync.dma_start(
                    out=w_sb[:],
                    in_=w_d.ap().rearrange("(cc p) d -> p cc d", p=128),
                )
            wp_sb = static.tile([128, C], bf16, tag="wp")
            nc.sync.dma_start(out=wp_sb[:], in_=wp_d[:])
            tril_sb = static.tile([128, 128], bf16, tag="tril")
            nc.sync.dma_start(out=tril_sb[:], in_=tril_d[:])
            ident_sb = static.tile([128, 128], bf16, tag="ident")
            nc.sync.dma_start(out=ident_sb[:], in_=ident_d[:])
            ones_sb = static.tile([128, 128], f32r, tag="ones")
            nc.sync.dma_start(out=ones_sb[:], in_=ones_d[:])
            bsel_sb = static.tile([65, 128], f32r, tag="bsel")
            nc.sync.dma_start(out=bsel_sb[:], in_=bsel_d[:])

            def phase_a_alloc():
                st = {}
                st["qts"], st["kts"] = [], []
                for _tb in range(NTB):
                    qt_t = qtp.tile([128, 512], bf16, tag="qt", name="qt_t")
                    kt_t = ktp.tile([128, 512], bf16, tag="kt", name="kt_t")
                    st["qts"].append(qt_t)
                    st["kts"].append(kt_t)
                st["vh0s"], st["vh1s"] = [], []
                for _kc in range(NKC):
                    vh0_t = vh0p.tile([128, 65], bf16, tag="vh0",
                                      name="vh0_t")
                    vh1_t = vh1p.tile([128, 128], bf16, tag="vh1",
                                      name="vh1_t")
                    # ones columns for the denominator trick
                    nc.vector.tensor_copy(vh0_t[:, 64:65], ones_sb[:, 0:1])
                    nc.vector.tensor_copy(vh1_t[:, 0:1], ones_sb[:, 0:1])
                    st["vh0s"].append(vh0_t)
                    st["vh1s"].append(vh1_t)
                return st

            def phase_a_tb(st, b, tb):
                qts, kts = st["qts"], st["kts"]
                vh0s, vh1s = st["vh0s"], st["vh1s"]
                xts = []
                for cc in range(CCH):
                    xt_t = xtp.tile([128, 512], bf16, tag="xt", name="xt_t")
                    nc.sync.dma_start(
                        out=xt_t[:],
                        in_=xT_d[b, cc * 128:(cc + 1) * 128,
                                 tb * 512:(tb + 1) * 512],
                    )
                    xts.append(xt_t)
                for which, w_sb in (("q", wq_sb), ("k", wk_sb),
                                    ("v", wv_sb)):
                    ps = mmps.tile([128, 512], f32, tag="mm", name="ps")
                    for cc in range(CCH):
                        nc.tensor.matmul(
                            ps[:], w_sb[:, cc, :], xts[cc][:],
                            start=(cc == 0), stop=(cc == CCH - 1),
                        )
                    if which == "q":
                        nc.vector.tensor_copy(qts[tb][:], ps[:])
                    elif which == "k":
                        nc.vector.tensor_copy(kts[tb][:], ps[:])
                    else:
                        vt_sb = vtp.tile([128, 512], bf16, tag="vt",
                                         name="vt_sb")
                        nc.vector.tensor_copy(vt_sb[:], ps[:])
                        for j in range(4):
                            kc = tb * 4 + j
                            tps = mmps.tile([128, 128], bf16, tag="mm",
                                            name="tps")
                            nc.tensor.transpose(
                                tps[:], vt_sb[:, j * 128:(j + 1) * 128],
                                ident_sb[:])
                            nc.vector.tensor_copy(
                                vh0s[kc][:, 0:64], tps[:, 0:64])
                            nc.vector.tensor_copy(
                                vh1s[kc][:, 64:128], tps[:, 64:128])

            def phase_b_qb(st, b, qb):
                qts, kts = st["qts"], st["kts"]
                vh0s, vh1s = st["vh0s"], st["vh1s"]
                yt_sb = ytp.tile([128, 512], bf16, tag="yt", name="yt_sb")
                nkc = 4 * qb + 4
                av0 = avps.tile([128, QB], f32, tag="av", name="av0")
                av1 = avps.tile([128, QB], f32, tag="av", name="av1")
                av = [av0, av1]
                for kc in range(nkc):
                    m = kc - 4 * qb  # >= 0 on diagonal chunks
                    lo = 0 if m < 0 else 128 * m
                    for h in range(2):
                        st_ps = sps.tile([128, QB], f32, tag="s",
                                         name="st_ps")
                        nc.tensor.matmul(
                            st_ps[:, lo:QB],
                            kts[kc // 4][64 * h:64 * h + 64,
                                         (kc % 4) * 128:
                                         (kc % 4) * 128 + 128],
                            qts[qb][64 * h:64 * h + 64, lo:QB],
                            start=True, stop=True,
                        )
                        et = etp.tile([128, QB], bf16, tag="et", name="et")
                        nc.scalar.activation(et[:, lo:QB], st_ps[:, lo:QB],
                                             EXP)
                        if m >= 0:
                            nc.vector.tensor_mul(
                                et[:, lo:lo + 128], et[:, lo:lo + 128],
                                tril_sb[:])
                        avt = av[0][0:65, lo:QB] if h == 0 \
                            else av[1][:, lo:QB]
                        lhsT = vh0s[kc][:] if h == 0 else vh1s[kc][:]
                        nc.tensor.matmul(
                            avt, lhsT, et[:, lo:QB],
                            start=(kc == 0), stop=(kc == nkc - 1),
                        )
                # denominators -> reciprocal -> normalized y^T
                dst = dstp.tile([65, QB], f32r, tag="dst", name="dst")
                nc.vector.tensor_copy(dst[64:65, :], av[0][64:65, :])
                nc.vector.tensor_copy(dst[0:1, :], av[1][0:1, :])
                bps = sps.tile([128, QB], f32, tag="s", name="bps")
                nc.tensor.matmul(bps[:], bsel_sb[:], dst[:, :],
                                 start=True, stop=True)
                rf = rfp.tile([128, QB], f32, tag="rf", name="rf")
                scr = rfp.tile([128, QB], f32, tag="scr", name="scr")
                nc.vector.reciprocal_approx_accurate(rf[:], bps[:], scr[:])
                for h in range(2):
                    src_av = av[0][0:64, :] if h == 0 else av[1][64:128, :]
                    rfs = rf[0:64, :] if h == 0 else rf[64:128, :]
                    nc.vector.tensor_mul(
                        yt_sb[64 * h:64 * h + 64, :], src_av, rfs)
                # output projection for this q-block's four t-chunks
                for j in range(4):
                    tchunk = qb * 4 + j
                    for n in range(2):
                        pps = mmps.tile([128, 512], f32, tag="mm",
                                        name="pps")
                        nc.tensor.matmul(
                            pps[:],
                            yt_sb[:, j * 128:(j + 1) * 128],
                            wp_sb[:, n * 512:(n + 1) * 512],
                            start=True, stop=True,
                        )
                        osb = osbp.tile([128, 512], f32, tag="osb",
                                        name="osb")
                        nc.vector.tensor_copy(osb[:], pps[:])
                        nc.sync.dma_start(
                            out=out_d[b, tchunk * 128:(tchunk + 1) * 128,
                                      n * 512:(n + 1) * 512],
                            in_=osb[:],
                        )

            # batch 0 QKV, then batch 0 attention interleaved with
            # batch 1 QKV (one t-block per q-block) to keep PE dense
            st0 = phase_a_alloc()
            for tb in range(NTB):
                phase_a_tb(st0, 0, tb)
            st1 = phase_a_alloc()
            for qb in range(NQB):
                phase_b_qb(st0, 0, qb)
                phase_a_tb(st1, 1, qb)
            for qb in range(NQB):
                phase_b_qb(st1, 1, qb)

    nc.compile()
    return nc


def _get_program():
    global _PROGRAM
    if _PROGRAM is None:
        _PROGRAM = _build_program()
    return _PROGRAM


def _make_in_maps(x, W_kqv, W_proj):
    import ml_dtypes

    bf = ml_dtypes.bfloat16
    xT = np.ascontiguousarray(x.transpose(0, 2, 1)).astype(bf)
    scale = 1.0 / math.sqrt(HS)

    ident = np.eye(128, dtype=np.float32).astype(bf)
    bsel = np.zeros((65, 128), dtype=np.float32)
    bsel[64, 0:64] = 1.0
    bsel[0, 64:128] = 1.0

    in_maps = []
    for c in range(NCORES):
        lo, hi = CPC * c, CPC * (c + 1)
        in_maps.append({
            "xT": xT,
            "wq": (np.ascontiguousarray(W_kqv[:, lo:hi]) * scale).astype(bf),
            "wk": np.ascontiguousarray(W_kqv[:, C + lo:C + hi]).astype(bf),
            "wv": np.ascontiguousarray(W_kqv[:, 2 * C + lo:2 * C + hi]).astype(bf),
            "wp": np.ascontiguousarray(W_proj[lo:hi, :]).astype(bf),
            "ident": ident,
            "tril": np.triu(np.ones((128, 128), dtype=np.float32)).astype(bf),
            "ones": np.ones((128, 128), dtype=np.float32),
            "bsel": bsel,
        })
    return in_maps


def run(x, W_kqv, W_proj, b_proj, trace=False, trace_cores=None):
    from concourse.bass_utils import run_bass_kernel_spmd

    nc = _get_program()
    in_maps = _make_in_maps(np.asarray(x, dtype=np.float32),
                            np.asarray(W_kqv, dtype=np.float32),
                            np.asarray(W_proj, dtype=np.float32))
    res = run_bass_kernel_spmd(
        nc, in_maps, list(range(NCORES)),
        trace=trace, trace_cores=trace_cores,
    )
    out = np.zeros((B, T, C), dtype=np.float32)
    for c in range(NCORES):
        out += res.results[c]["out"]
    out += np.asarray(b_proj, dtype=np.float32)
    return out, res


def kernel(x, W_kqv, W_proj, b_proj):
    out, _ = run(x, W_kqv, W_proj, b_proj)
    return out


# revision 37
# speedup vs baseline: 1.2476x; 1.0659x over previous
"""Causal multi-head attention block on 8 trn2 NeuronCores.

Tensor-parallel over heads: core c handles heads 2c, 2c+1 (for both batch
rows), computing q/k/v projections for its 128 channels, causal softmax
attention, and a partial output projection against its 128 rows of W_proj.
The host sums the 8 partial outputs and adds b_proj.

Layout strategy: everything on-chip lives in "transposed" space. The host
supplies x^T [B, C, T]; Q^T/K^T/V^T [128, T] come from matmuls with the
weight chunk as the stationary operand. V is PE-transposed back to [T, 128]
and augmented with a ones column so the P@V matmul also yields the softmax
denominator. Scores are computed as S^T = (K^T)^T @ Q^T with the two heads
row-packed on the PE array (each K=64). Causal masking adds a -60 constant
via an extra accumulating matmul (identity @ mask). All matmuls are fp32r
with free dim 512 (full-rate). Normalization is a broadcast-DMA of the
denominator row + reciprocal + multiply fused into the y^T copy.
"""

import math

import numpy as np

B, T, C, H = 2, 2048, 1024, 16
HS = C // H  # 64 head size
NCORES = 8
HPC = H // NCORES  # 2 heads per core
CPC = HPC * HS  # 128 channels per core
QB = 512  # q block width
NQB = T // QB  # 4
NKC = T // 128  # 16 k-chunks
CCH = C // 128  # 8 contraction chunks
NTB = T // 512  # 4 t-blocks for QKV
MASK_NEG = -60.0

_PROGRAM = None


def _build_program():
    import concourse.bass as bass
    import concourse.tile as tile
    from concourse import bacc, mybir

    f32 = mybir.dt.float32
    f32r = mybir.dt.float32r
    bf16 = mybir.dt.bfloat16
    EXP = mybir.ActivationFunctionType.Exp
    LN = mybir.ActivationFunctionType.Ln

    nc = bacc.Bacc("TRN2", target_bir_lowering=False, debug=False,
                   num_devices=NCORES)

    xT_d = nc.dram_tensor("xT", [B, C, T], bf16, kind="ExternalInput")
    wq_d = nc.dram_tensor("wq", [C, CPC], bf16, kind="ExternalInput")
    wk_d = nc.dram_tensor("wk", [C, CPC], bf16, kind="ExternalInput")
    wv_d = nc.dram_tensor("wv", [C, CPC], bf16, kind="ExternalInput")
    wp_d = nc.dram_tensor("wp", [CPC, C], bf16, kind="ExternalInput")
    ident_d = nc.dram_tensor("ident", [128, 128], bf16, kind="ExternalInput")
    tril_d = nc.dram_tensor("tril", [128, 128], bf16, kind="ExternalInput")
    ones_d = nc.dram_tensor("ones", [128, 128], f32r, kind="ExternalInput")
    bsel_d = nc.dram_tensor("bsel", [65, 128], f32r, kind="ExternalInput")
    out_d = nc.dram_tensor("out", [B, T, C], f32, kind="ExternalOutput")

    with tile.TileContext(nc) as tc:
        with (
            tc.tile_pool(name="static", bufs=1) as static,
            tc.tile_pool(name="xt", bufs=16) as xtp,
            tc.tile_pool(name="qt", bufs=10) as qtp,
            tc.tile_pool(name="kt", bufs=10) as ktp,
            tc.tile_pool(name="vt", bufs=3) as vtp,
            tc.tile_pool(name="vh0", bufs=34) as vh0p,
            tc.tile_pool(name="vh1", bufs=34) as vh1p,
            tc.tile_pool(name="et", bufs=4) as etp,
            tc.tile_pool(name="yt", bufs=3) as ytp,
            tc.tile_pool(name="dst", bufs=3) as dstp,
            tc.tile_pool(name="rf", bufs=4) as rfp,
            tc.tile_pool(name="osb", bufs=6) as osbp,
            tc.tile_pool(name="mm", bufs=2, space="PSUM") as mmps,
            tc.tile_pool(name="sp", bufs=4, space="PSUM") as sps,
            tc.tile_pool(name="av", bufs=2, space="PSUM") as avps,
        ):
            # ---- static loads ----
            wq_sb = static.tile([128, CCH, CPC], bf16, tag="wq")
            wk_sb = static.tile([128, CCH, CPC], bf16, tag="wk")
            wv_sb = static.tile([128, CCH, CPC], bf16, tag="wv")
            for w_sb, w_d in ((wq_sb, wq_d), (wk_sb, wk_d), (wv_sb, wv_d)):
                nc.sync.dma_start(
                    out=w_sb[:],
                    in_=w_d.ap().rearrange("(cc p) d -> p cc d", p=128),
                )
            wp_sb = static.tile([128, C], bf16, tag="wp")
            nc.sync.dma_start(out=wp_sb[:], in_=wp_d[:])
            tril_sb = static.tile([128, 128], bf16, tag="tril")
            nc.sync.dma_start(out=tril_sb[:], in_=tril_d[:])
            ident_sb = static.tile([128, 128], bf16, tag="ident")
            nc.sync.dma_start(out=ident_sb[:], in_=ident_d[:])
            ones_sb = static.tile([128, 128], f32r, tag="ones")
            nc.sync.dma_start(out=ones_sb[:], in_=ones_d[:])
            bsel_sb = static.tile([65, 128], f32r, tag="bsel")
            nc.sync.dma_start(out=bsel_sb[:], in_=bsel_d[:])

            def phase_a_alloc():
                st = {}
                st["qts"], st["kts"] = [], []
                for _tb in range(NTB):
                    qt_t = qtp.tile([128, 512], bf16, tag="qt", name="qt_t")
                    kt_t = ktp.tile([128, 512], bf16, tag="kt", name="kt_t")
                    st["qts"].append(qt_t)
                    st["kts"].append(kt_t)
                st["vh0s"], st["vh1s"] = [], []
                for _kc in range(NKC):
                    vh0_t = vh0p.tile([128, 65], bf16, tag="vh0",
                                      name="vh0_t")
                    vh1_t = vh1p.tile([128, 128], bf16, tag="vh1",
                                      name="vh1_t")
                    # ones columns for the denominator trick
                    nc.vector.tensor_copy(vh0_t[:, 64:65], ones_sb[:, 0:1])
                    nc.vector.tensor_copy(vh1_t[:, 0:1], ones_sb[:, 0:1])
                    st["vh0s"].append(vh0_t)
                    st["vh1s"].append(vh1_t)
                return st

            def phase_a_tb(st, b, tb):
                qts, kts = st["qts"], st["kts"]
                vh0s, vh1s = st["vh0s"], st["vh1s"]
                xts = []
                for cc in range(CCH):
                    xt_t = xtp.tile([128, 512], bf16, tag="xt", name="xt_t")
                    nc.sync.dma_start(
                        out=xt_t[:],
                        in_=xT_d[b, cc * 128:(cc + 1) * 128,
                                 tb * 512:(tb + 1) * 512],
                    )
                    xts.append(xt_t)
                for which, w_sb in (("q", wq_sb), ("k", wk_sb),
                                    ("v", wv_sb)):
                    ps = mmps.tile([128, 512], f32, tag="mm", name="ps")
                    for cc in range(CCH):
                        nc.tensor.matmul(
                            ps[:], w_sb[:, cc, :], xts[cc][:],
                            start=(cc == 0), stop=(cc == CCH - 1),
                        )
                    if which == "q":
                        nc.vector.tensor_copy(qts[tb][:], ps[:])
                    elif which == "k":
                        nc.vector.tensor_copy(kts[tb][:], ps[:])
                    else:
                        vt_sb = vtp.tile([128, 512], bf16, tag="vt",
                                         name="vt_sb")
                        nc.vector.tensor_copy(vt_sb[:], ps[:])
                        for j in range(4):
                            kc = tb * 4 + j
                            tps = mmps.tile([128, 128], bf16, tag="mm",
                                            name="tps")
                            nc.tensor.transpose(
                                tps[:], vt_sb[:, j * 128:(j + 1) * 128],
                                ident_sb[:])
                            nc.vector.tensor_copy(
                                vh0s[kc][:, 0:64], tps[:, 0:64])
                            nc.vector.tensor_copy(
                                vh1s[kc][:, 64:128], tps[:, 64:128])

            def phase_b_qb(st, b, qb):
                qts, kts = st["qts"], st["kts"]
                vh0s, vh1s = st["vh0s"], st["vh1s"]
                yt_sb = ytp.tile([128, 512], bf16, tag="yt", name="yt_sb")
                nkc = 4 * qb + 4
                av0 = avps.tile([128, QB], f32, tag="av", name="av0")
                av1 = avps.tile([128, QB], f32, tag="av", name="av1")
                av = [av0, av1]
                for kc in range(nkc):
                    m = kc - 4 * qb  # >= 0 on diagonal chunks
                    lo = 0 if m < 0 else 128 * m
                    for h in range(2):
                        st_ps = sps.tile([128, QB], f32, tag="s",
                                         name="st_ps")
                        nc.tensor.matmul(
                            st_ps[:, lo:QB],
                            kts[kc // 4][64 * h:64 * h + 64,
                                         (kc % 4) * 128:
                                         (kc % 4) * 128 + 128],
                            qts[qb][64 * h:64 * h + 64, lo:QB],
                            start=True, stop=True,
                        )
                        et = etp.tile([128, QB], bf16, tag="et", name="et")
                        nc.scalar.activation(et[:, lo:QB], st_ps[:, lo:QB],
                                             EXP)
                        if m >= 0:
                            nc.gpsimd.tensor_mul(
                                et[:, lo:lo + 128], et[:, lo:lo + 128],
                                tril_sb[:])
                        avt = av[0][0:65, lo:QB] if h == 0 \
                            else av[1][:, lo:QB]
                        lhsT = vh0s[kc][:] if h == 0 else vh1s[kc][:]
                        nc.tensor.matmul(
                            avt, lhsT, et[:, lo:QB],
                            start=(kc == 0), stop=(kc == nkc - 1),
                        )
                # denominators -> reciprocal -> normalized y^T
                dst = dstp.tile([65, QB], f32r, tag="dst", name="dst")
                nc.vector.tensor_copy(dst[64:65, :], av[0][64:65, :])
                nc.vector.tensor_copy(dst[0:1, :], av[1][0:1, :])
                bps = sps.tile([128, QB], f32, tag="s", name="bps")
                nc.tensor.matmul(bps[:], bsel_sb[:], dst[:, :],
                                 start=True, stop=True)
                rf = rfp.tile([128, QB], f32, tag="rf", name="rf")
                nc.vector.reciprocal_approx_fast(rf[:], bps[:])
                for h in range(2):
                    src_av = av[0][0:64, :] if h == 0 else av[1][64:128, :]
                    rfs = rf[0:64, :] if h == 0 else rf[64:128, :]
                    nc.vector.tensor_mul(
                        yt_sb[64 * h:64 * h + 64, :], src_av, rfs)
                # output projection for this q-block's four t-chunks
                for j in range(4):
                    tchunk = qb * 4 + j
                    for n in range(2):
                        pps = mmps.tile([128, 512], f32, tag="mm",
                                        name="pps")
                        nc.tensor.matmul(
                            pps[:],
                            yt_sb[:, j * 128:(j + 1) * 128],
                            wp_sb[:, n * 512:(n + 1) * 512],
                            start=True, stop=True,
                        )
                        osb = osbp.tile([128, 512], f32, tag="osb",
                                        name="osb")
                        nc.vector.tensor_copy(osb[:], pps[:])
                        nc.sync.dma_start(
                            out=out_d[b, tchunk * 128:(tchunk + 1) * 128,
                                      n * 512:(n + 1) * 512],
                            in_=osb[:],
                        )

            # batch 0 QKV, then batch 0 attention interleaved with
            # batch 1 QKV (one t-block per q-block) to keep PE dense
            st0 = phase_a_alloc()
            for tb in range(NTB):
                phase_a_tb(st0, 0, tb)
            st1 = phase_a_alloc()
            for qb in range(NQB):
                phase_b_qb(st0, 0, qb)
                phase_a_tb(st1, 1, qb)
            for qb in range(NQB):
                phase_b_qb(st1, 1, qb)

    nc.compile()
    return nc


def _get_program():
    global _PROGRAM
    if _PROGRAM is None:
        _PROGRAM = _build_program()
    return _PROGRAM


def _make_in_maps(x, W_kqv, W_proj):
    import ml_dtypes

    bf = ml_dtypes.bfloat16
    xT = np.ascontiguousarray(x.transpose(0, 2, 1)).astype(bf)
    scale = 1.0 / math.sqrt(HS)

    ident = np.eye(128, dtype=np.float32).astype(bf)
    bsel = np.zeros((65, 128), dtype=np.float32)
    bsel[64, 0:64] = 1.0
    bsel[0, 64:128] = 1.0

    in_maps = []
    for c in range(NCORES):
        lo, hi = CPC * c, CPC * (c + 1)
        in_maps.append({
            "xT": xT,
            "wq": (np.ascontiguousarray(W_kqv[:, lo:hi]) * scale).astype(bf),
            "wk": np.ascontiguousarray(W_kqv[:, C + lo:C + hi]).astype(bf),
            "wv": np.ascontiguousarray(W_kqv[:, 2 * C + lo:2 * C + hi]).astype(bf),
            "wp": np.ascontiguousarray(W_proj[lo:hi, :]).astype(bf),
            "ident": ident,
            "tril": np.triu(np.ones((128, 128), dtype=np.float32)).astype(bf),
            "ones": np.ones((128, 128), dtype=np.float32),
            "bsel": bsel,
        })
    return in_maps


def run(x, W_kqv, W_proj, b_proj, trace=False, trace_cores=None):
    from concourse.bass_utils import run_bass_kernel_spmd

    nc = _get_program()
    in_maps = _make_in_maps(np.asarray(x, dtype=np.float32),
                            np.asarray(W_kqv, dtype=np.float32),
                            np.asarray(W_proj, dtype=np.float32))
    res = run_bass_kernel_spmd(
        nc, in_maps, list(range(NCORES)),
        trace=trace, trace_cores=trace_cores,
    )
    out = np.zeros((B, T, C), dtype=np.float32)
    for c in range(NCORES):
        out += res.results[c]["out"]
    out += np.asarray(b_proj, dtype=np.float32)
    return out, res


def kernel(x, W_kqv, W_proj, b_proj):
    out, _ = run(x, W_kqv, W_proj, b_proj)
    return out
